# revision 28
# baseline (speedup 1.0000x reference)
"""Trainium2 Bass kernel for nn_Ensemble_FC (BatchEnsemble fully-connected layer).

Math (reference):
    emb   = relu(alpha @ enc1_w.T + enc1_b)          # (M, H)
    mu    = emb @ encm_w.T + encm_b                  # (M, H)
    z     = eps * exp(0.5 * mu) + mu
    adec  = z @ dec_w.T + dec_b                      # (M, IN)
    out[m*B+i, o] = (sum_k x[i,k] * adec[m,k] * fc_w[o,k]) * gamma[m,o] + bias_p[m,o]

The VAE encoder producing adec is 4x33x4096 ~ 1.1 MFLOP of the 68.7 GFLOP
problem (0.002%); it is folded into host-side input preparation (like the
dec_b / 0.5*encm_b constant folding this kernel always did), so the device
program is the pure BatchEnsemble GEMM.

Sharding: tensor-parallel column-split of fc_w / gamma / bias_p over
out_features (4096 -> 8 x 512).  Every core runs the full (M*B = 2048)-row
GEMM for its 512 output columns.

On-chip layout is transposed ([feature, row]) so per-model scales
(adec, gamma, bias) are per-partition scalars:
    out_core[o_local, m*B+i] = psum * gamma + bias,
    psum = sum_kc  wT[kc, o-chunk].T @ (xT[kc] * adecT[kc, m])
GEMM runs in bf16 (rounded on host), fp32 PSUM accumulation; epilogue
scale+bias in fp32, stored bf16 and upcast on host (tolerance 2e-2).

Perf structure (trace-driven):
- ~7.3us fixed prologue (runtime barriers + IRAM loads) before any user
  instruction runs; warm-up matmuls bridge from there to first data and
  trip the HAM clock gate (cold PE 1.2 GHz -> 2.4 GHz after ~3.4us busy).
- Two m-phases of the GEMM, each k-outer over all 4 output chunks so each
  scaled activation tile xab(k,m) is built ONCE on the DVE and feeds 4
  matmuls (the old 2-pass oc-split built every xab twice and the DVE
  FIFO head-of-line blocked the stream).
- Phase 2 xab prefetch is dep-pinned behind the same-k phase-1 xab so a
  not-yet-landed DMA group can never head-of-line block the DVE queue.
- Phase 2 matmul chains run slot-contiguous (32 MMs per (oc,m)), so PSUM
  bank recycling pipelines against phase-1 epilogues with one ~0.8us
  bubble; phase-1 stays k-outer interleaved to track bulk-DMA arrival.
- Bulk DMA: tiny head groups + few big tail groups (issues are
  semaphore-chained per queue), and x's 1MB tail groups are dep-pinned
  behind w head-group completions — the two rings share the SDMA engine
  pool and the w ring otherwise ramps too slowly for its k=2..15
  deadlines while x runs 2x ahead of demand.
- Measured (healthy 2.4 GHz P-state): ~131us vs the 142us 2-pass
  baseline; stream runs at the 216ns/MM N=512 bf16 floor throughout.
"""

import os
import sys

for _p in ("/opt/trn_rl_repo",):
    if os.path.isdir(_p) and _p not in sys.path:
        sys.path.insert(0, _p)

import numpy as np
import ml_dtypes

import concourse.bass as bass  # noqa: F401  (registers engine libraries)
import concourse.mybir as mybir
import concourse.tile as tile
from concourse import bacc
from concourse.bass_utils import run_bass_kernel_spmd

N_CORES = 8
M = 4          # ensemble members
B = 512        # batch
IN = 4096      # in_features (contraction)
OUT = 4096     # out_features
H = 32         # encoder hidden
P = 128        # partitions
KC = IN // P   # 32 contraction chunks of 128
O_CORE = OUT // N_CORES   # 512 output columns per core
OC = O_CORE // P          # 4 o-chunks of 128 per core
N_WARM = 10    # PE warm-up matmuls bridging the prologue to first data;
               # sized so the PE is continuously busy from the prologue
               # into the stream (any idle gap resets the HAM busy window
               # and the first ~12 stream matmuls run at 1.2 GHz)

# bulk-stream DMA groups (kc each); small head groups so the first
# matmuls aren't gated on a full 512KB transfer
# issues are semaphore-chained per queue (~3 in flight), so use few big
# tail groups: small heads start the stream early, big tails keep the
# issue pipeline from gating delivery
GROUP_KCS = [1, 1, 2, 4, 8, 8, 8]
G = len(GROUP_KCS)
GROUP_OF_K = []
for _g, _n in enumerate(GROUP_KCS):
    GROUP_OF_K += [(_g, _j) for _j in range(_n)]

# gb32 column layout (f32, [128, GB_W])
GB_G = 0                      # [p, oc, m]  OC*M = 16
GB_B = GB_G + OC * M
GB_W = GB_B + OC * M          # 32

F32 = mybir.dt.float32
BF16 = mybir.dt.bfloat16
AF = mybir.ActivationFunctionType

_nc_cache = {}


def _build_nc():
    """Build and compile the per-core Bass/Tile program (SPMD, same on all 8)."""
    nc = bacc.Bacc("TRN2", num_devices=N_CORES, debug=False)

    xh_d = nc.declare_dram_parameter("xh", [P, KC, B], BF16, isOutput=False)
    wh_d = nc.declare_dram_parameter("wh", [P, KC, O_CORE], BF16, isOutput=False)
    ad32_d = nc.declare_dram_parameter("ad32", [P, KC * M], F32, isOutput=False)
    gb32_d = nc.declare_dram_parameter("gb32", [P, GB_W], F32, isOutput=False)
    out_d = nc.declare_dram_parameter("out", [O_CORE, M * B], BF16, isOutput=True)

    with tile.TileContext(nc) as tc:
        with (
            tc.tile_pool(name="consts", bufs=1) as consts,
            tc.tile_pool(name="xt", bufs=G) as xt_pool,
            tc.tile_pool(name="wt", bufs=G) as wt_pool,
            tc.tile_pool(name="xab1", bufs=16) as xab1_pool,
            tc.tile_pool(name="xab2", bufs=2 * KC) as xab2_pool,
            tc.tile_pool(name="ps", bufs=8, space="PSUM") as ps_pool,
            tc.tile_pool(name="osb", bufs=6) as out_pool,
        ):
            # ---- PE warm-up: garbage matmuls bridge the prologue + DMA
            # latency and trip the HAM activity monitor (1.2 -> 2.4 GHz).
            wu_src = consts.tile([P, B], BF16)
            nc.gpsimd.memset(wu_src[:], 0.0)
            wu_ps = ps_pool.tile([P, B], F32, tag="ps")
            for i in range(N_WARM):
                nc.tensor.matmul(
                    wu_ps[:], lhsT=wu_src[:, :P], rhs=wu_src[:], start=True, stop=True
                )

            # ---- DMA issue.  Tiny constants (80KB) ride the ACT ring; bulk
            # x on the SP ring, bulk w on the Pool SWDGE ring.  The rings
            # round-robin at packet granularity, so the constants finish
            # fast even against the bulk flood — no hold-back needed.
            ad32_sb = consts.tile([P, KC * M], F32)
            nc.scalar.dma_start(ad32_sb[:], ad32_d.ap())
            gb32_sb = consts.tile([P, GB_W], F32)
            nc.scalar.dma_start(gb32_sb[:], gb32_d.ap())
            xt_tiles = []
            wt_tiles = []
            xdmas = []
            wdmas = []
            k0 = 0
            for g in range(G):
                ks = slice(k0, k0 + GROUP_KCS[g])
                k0 += GROUP_KCS[g]
                xt = xt_pool.tile([P, GROUP_KCS[g], B], BF16, tag="xt")
                xdmas.append(nc.sync.dma_start(xt[:], xh_d.ap()[:, ks, :]))
                wt = wt_pool.tile([P, GROUP_KCS[g], O_CORE], BF16, tag="wt")
                wdmas.append(nc.gpsimd.dma_start(wt[:], wh_d.ap()[:, ks, :]))
                xt_tiles.append(xt)
                wt_tiles.append(wt)
            # The two bulk rings share the SDMA engine pool, and x's head
            # delivery runs ~2x faster than consumption while w's ramps too
            # slowly to make its k=4..15 deadlines.  Stagger x's 1MB tail
            # groups behind w's head completions so w gets the engine share
            # while its deadlines are tight; x's tails still land with
            # >10us of slack.
            for xg, wg in ((4, 1), (5, 4), (6, 5)):
                tile.add_dep_helper(
                    xdmas[xg].ins, wdmas[wg].ins,
                    reason=f"x tail g{xg} yields SDMA share to w head g{wg}",
                )

            # consume the warm-up psum (keeps bacc DCE honest) on the
            # otherwise-idle ACT engine so the DVE stream is untouched;
            # this also frees the wu PSUM bank for phase-1's 8th chain.
            wu_sink = consts.tile([P, 32], F32)
            nc.scalar.activation(wu_sink[:], wu_ps[:, :32], AF.Copy)

            g_v = gb32_sb[:, GB_G:GB_B].rearrange("p (o m) -> p o m", m=M)
            b_v = gb32_sb[:, GB_B:GB_W].rearrange("p (o m) -> p o m", m=M)

            def epilogue(ps, oc, m, name, on_dve=False):
                osb = out_pool.tile([P, B], BF16, tag="osb", name=name)
                if on_dve:
                    # fused (psum * gamma) + bias on the DVE (~2x faster
                    # than ACT) — used for the tail-exposed final slot
                    nc.vector.tensor_scalar(
                        osb[:], ps[:],
                        g_v[:, oc, m : m + 1], b_v[:, oc, m : m + 1],
                        mybir.AluOpType.mult, mybir.AluOpType.add,
                    )
                else:
                    nc.scalar.activation(
                        osb[:],
                        ps[:],
                        AF.Identity,
                        bias=b_v[:, oc, m : m + 1],
                        scale=g_v[:, oc, m : m + 1],
                    )
                nc.sync.dma_start(
                    out_d.ap()[oc * P : (oc + 1) * P, m * B : (m + 1) * B],
                    osb[:],
                )

            # ---- main GEMM: two m-phases, 8 PSUM chains each.
            xab1_ref = {}
            for ph, ms in enumerate(((0, 1), (2, 3))):
                pss = {}
                for m in ms:
                    for oc in range(OC):
                        pss[(oc, m)] = ps_pool.tile(
                            [P, B], F32, tag="ps", name=f"ps{ph}_{m}_{oc}"
                        )
                xabs = {}
                if ph == 0:
                    # k-outer, all 8 chains interleaved: consumption tracks
                    # the bulk-DMA arrival rate without stalling.
                    for k in range(KC):
                        g, j = GROUP_OF_K[k]
                        for m in ms:
                            xab = xab1_pool.tile(
                                [P, B], BF16, tag="xab1", name=f"xa{ph}_{k}_{m}"
                            )
                            op = nc.vector.tensor_scalar_mul(
                                xab[:], xt_tiles[g][:, j, :],
                                ad32_sb[:, k * M + m : k * M + m + 1],
                            )
                            xab1_ref[k] = op
                            xabs[(k, m)] = xab
                        for oc in range(OC):
                            for m in ms:
                                nc.tensor.matmul(
                                    pss[(oc, m)][:],
                                    lhsT=wt_tiles[g][:, j, oc * P : (oc + 1) * P],
                                    rhs=xabs[(k, m)][:],
                                    start=(k == 0),
                                    stop=(k == KC - 1),
                                )
                else:
                    # all data resident.  Phase-2 xab prefetch runs on the
                    # otherwise-idle gpsimd engine (m-major so the m2 set
                    # finishes long before its chains need it): the DVE then
                    # only carries the 2-per-k phase-1 stream at ~40% duty
                    # and can never fall behind the matmul stream, and a
                    # not-yet-landed DMA group can only HOL-block other
                    # prefetches, which have huge deadline slack.  Chains
                    # run slot-contiguous so PSUM bank recycling pipelines
                    # against phase-1 epilogues.
                    for m in ms:
                        for k in range(KC):
                            g, j = GROUP_OF_K[k]
                            xab = xab2_pool.tile(
                                [P, B], BF16, tag="xab2", name=f"xa{ph}_{k}_{m}"
                            )
                            nc.gpsimd.tensor_scalar_mul(
                                xab[:], xt_tiles[g][:, j, :],
                                ad32_sb[:, k * M + m : k * M + m + 1],
                            )
                            xabs[(k, m)] = xab
                    for m in ms:
                        for oc in range(OC):
                            for k in range(KC):
                                g, j = GROUP_OF_K[k]
                                nc.tensor.matmul(
                                    pss[(oc, m)][:],
                                    lhsT=wt_tiles[g][:, j, oc * P : (oc + 1) * P],
                                    rhs=xabs[(k, m)][:],
                                    start=(k == 0),
                                    stop=(k == KC - 1),
                                )
                # epilogues in the same slot order phase 2 allocates PSUM,
                # so the pool ring pairs each phase-2 chain with the
                # phase-1 chain whose epilogue it waits on.
                for m in ms:
                    for oc in range(OC):
                        last = ph == 1 and m == ms[-1] and oc == OC - 1
                        epilogue(pss[(oc, m)], oc, m, f"o{ph}_{m}_{oc}",
                                 on_dve=last)

    nc.compile()
    return nc


def _get_nc():
    if "nc" not in _nc_cache:
        _nc_cache["nc"] = _build_nc()
    return _nc_cache["nc"]


def _pk(a2d):
    """(C*P, W) -> (P, C*W): row 128c+p -> [p, c, :] flattened."""
    c = a2d.shape[0] // P
    w = a2d.shape[1]
    return np.ascontiguousarray(
        a2d.reshape(c, P, w).transpose(1, 0, 2).reshape(P, c * w)
    )


def kernel(
    x, eps, alpha, gamma, bias_p, fc_w,
    enc1_w, enc1_b, encm_w, encm_b, dec_w, dec_b,
):
    bf16 = ml_dtypes.bfloat16
    f32 = np.float32
    asc = np.ascontiguousarray

    x = np.asarray(x, f32)
    fc_w = np.asarray(fc_w, f32)

    # ---- fold the tiny VAE encoder into input prep (fp32, exact)
    alpha = np.asarray(alpha, f32)
    emb = np.maximum(alpha @ np.asarray(enc1_w, f32).T + np.asarray(enc1_b, f32), 0.0)
    mu = emb @ np.asarray(encm_w, f32).T + np.asarray(encm_b, f32)
    z = np.asarray(eps, f32) * np.exp(0.5 * mu) + mu
    adec = z @ np.asarray(dec_w, f32).T + np.asarray(dec_b, f32)   # (M, IN)

    # x: (B, IN) -> xh (P, KC, B) bf16, xh[p,k,r] = x[r, 128k+p]
    xh = asc(x.astype(bf16).T.reshape(KC, P, B).transpose(1, 0, 2))
    # fc_w: (OUT, IN) -> per-core wh (P, KC, O_CORE) bf16
    wT_full = fc_w.astype(bf16).T  # (IN, OUT) view

    ad32 = _pk(asc(adec.T))                               # (P, KC*M) f32

    gT_full = np.asarray(gamma, f32).T                    # (OUT, M)
    bT_full = np.asarray(bias_p, f32).T                   # (OUT, M)

    in_maps = []
    for c in range(N_CORES):
        o0, o1 = c * O_CORE, (c + 1) * O_CORE
        wh = asc(wT_full[:, o0:o1].reshape(KC, P, O_CORE).transpose(1, 0, 2))
        gb32 = np.empty((P, GB_W), f32)
        gb32[:, GB_G:GB_B] = _pk(asc(gT_full[o0:o1]))
        gb32[:, GB_B:GB_W] = _pk(asc(bT_full[o0:o1]))
        in_maps.append({"xh": xh, "wh": wh, "ad32": ad32, "gb32": gb32})

    nc = _get_nc()
    res = None
    for attempt in range(3):
        try:
            res = run_bass_kernel_spmd(nc, in_maps, list(range(N_CORES)))
            break
        except Exception:
            # transient NRT_EXEC_UNIT_UNRECOVERABLE wedges can follow an
            # earlier crashed process on the same cores; retry clears it
            if attempt == 2:
                raise
            import time

            time.sleep(5.0)
    outT = np.concatenate(
        [np.asarray(res.results[c]["out"]) for c in range(N_CORES)], axis=0
    )  # (OUT, M*B) bf16
    return asc(outT.T.astype(np.float32))  # (M*B, OUT)


# revision 29
# speedup vs baseline: 4.0535x; 4.0535x over previous
"""Trainium2 Bass kernel for nn_Ensemble_FC (BatchEnsemble fully-connected layer).

Math (reference):
    emb   = relu(alpha @ enc1_w.T + enc1_b)          # (M, H)
    mu    = emb @ encm_w.T + encm_b                  # (M, H)
    z     = eps * exp(0.5 * mu) + mu
    adec  = z @ dec_w.T + dec_b                      # (M, IN)
    out[m*B+i, o] = (sum_k x[i,k] * adec[m,k] * fc_w[o,k]) * gamma[m,o] + bias_p[m,o]

The VAE encoder producing adec is 4x33x4096 ~ 1.1 MFLOP of the 68.7 GFLOP
problem (0.002%); it is folded into host-side input preparation (like the
dec_b / 0.5*encm_b constant folding this kernel always did), so the device
program is the pure BatchEnsemble GEMM.

Sharding: tensor-parallel column-split of fc_w / gamma / bias_p over
out_features (4096 -> 8 x 512).  Every core runs the full (M*B = 2048)-row
GEMM for its 512 output columns.

On-chip layout is transposed ([feature, row]) so per-model scales
(adec, gamma, bias) are per-partition scalars:
    out_core[o_local, m*B+i] = psum * gamma + bias,
    psum = sum_kc  wT[kc, o-chunk].T @ (xT[kc] * adecT[kc, m])
GEMM runs in bf16 (rounded on host), fp32 PSUM accumulation; epilogue
scale+bias in fp32, stored bf16 and upcast on host (tolerance 2e-2).

Perf structure (trace-driven):
- ~7.3us fixed prologue (runtime barriers + IRAM loads) before any user
  instruction runs; warm-up matmuls bridge from there to first data and
  trip the HAM clock gate (cold PE 1.2 GHz -> 2.4 GHz after ~3.4us busy).
- Two m-phases of the GEMM, each k-outer over all 4 output chunks so each
  scaled activation tile xab(k,m) is built ONCE on the DVE and feeds 4
  matmuls (the old 2-pass oc-split built every xab twice and the DVE
  FIFO head-of-line blocked the stream).
- Phase 2 xab prefetch is dep-pinned behind the same-k phase-1 xab so a
  not-yet-landed DMA group can never head-of-line block the DVE queue.
- Phase 2 matmul chains run slot-contiguous (32 MMs per (oc,m)), so PSUM
  bank recycling pipelines against phase-1 epilogues with one ~0.8us
  bubble; phase-1 stays k-outer interleaved to track bulk-DMA arrival.
- Bulk DMA: tiny head groups + few big tail groups (issues are
  semaphore-chained per queue), and x's 1MB tail groups are dep-pinned
  behind w head-group completions — the two rings share the SDMA engine
  pool and the w ring otherwise ramps too slowly for its k=2..15
  deadlines while x runs 2x ahead of demand.
- Measured (healthy 2.4 GHz P-state): ~131us vs the 142us 2-pass
  baseline; stream runs at the 216ns/MM N=512 bf16 floor throughout.
"""

import os
import sys

for _p in ("/opt/trn_rl_repo",):
    if os.path.isdir(_p) and _p not in sys.path:
        sys.path.insert(0, _p)

import numpy as np
import ml_dtypes

import concourse.bass as bass  # noqa: F401  (registers engine libraries)
import concourse.mybir as mybir
import concourse.tile as tile
from concourse import bacc
from concourse.bass_utils import run_bass_kernel_spmd

N_CORES = 8
M = 4          # ensemble members
B = 512        # batch
IN = 4096      # in_features (contraction)
OUT = 4096     # out_features
H = 32         # encoder hidden
P = 128        # partitions
KC = IN // P   # 32 contraction chunks of 128
O_CORE = OUT // N_CORES   # 512 output columns per core
OC = O_CORE // P          # 4 o-chunks of 128 per core
N_WARM = 10    # PE warm-up matmuls bridging the prologue to first data;
               # sized so the PE is continuously busy from the prologue
               # into the stream (any idle gap resets the HAM busy window
               # and the first ~12 stream matmuls run at 1.2 GHz)

# bulk-stream DMA groups (kc each); small head groups so the first
# matmuls aren't gated on a full 512KB transfer
# issues are semaphore-chained per queue (~3 in flight), so use few big
# tail groups: small heads start the stream early, big tails keep the
# issue pipeline from gating delivery
GROUP_KCS = [1, 1, 2, 4, 8, 8, 8]
G = len(GROUP_KCS)
GROUP_OF_K = []
for _g, _n in enumerate(GROUP_KCS):
    GROUP_OF_K += [(_g, _j) for _j in range(_n)]

# gb32 column layout (f32, [128, GB_W])
GB_G = 0                      # [p, oc, m]  OC*M = 16
GB_B = GB_G + OC * M
GB_W = GB_B + OC * M          # 32

F32 = mybir.dt.float32
BF16 = mybir.dt.bfloat16
AF = mybir.ActivationFunctionType

_nc_cache = {}


def _build_nc():
    """Build and compile the per-core Bass/Tile program (SPMD, same on all 8)."""
    nc = bacc.Bacc("TRN2", num_devices=N_CORES, debug=False)

    xh_d = nc.declare_dram_parameter("xh", [P, KC, B], BF16, isOutput=False)
    wh_d = nc.declare_dram_parameter("wh", [P, KC, O_CORE], BF16, isOutput=False)
    ad32_d = nc.declare_dram_parameter("ad32", [P, KC * M], F32, isOutput=False)
    gb32_d = nc.declare_dram_parameter("gb32", [P, GB_W], F32, isOutput=False)
    out_d = nc.declare_dram_parameter("out", [O_CORE, M * B], BF16, isOutput=True)

    with tile.TileContext(nc) as tc:
        with (
            tc.tile_pool(name="consts", bufs=1) as consts,
            tc.tile_pool(name="xt", bufs=G) as xt_pool,
            tc.tile_pool(name="wt", bufs=G) as wt_pool,
            tc.tile_pool(name="xab1", bufs=16) as xab1_pool,
            tc.tile_pool(name="xab2", bufs=2 * KC) as xab2_pool,
            tc.tile_pool(name="ps", bufs=8, space="PSUM") as ps_pool,
            tc.tile_pool(name="osb", bufs=6) as out_pool,
        ):
            # ---- PE warm-up: garbage matmuls bridge the prologue + DMA
            # latency and trip the HAM activity monitor (1.2 -> 2.4 GHz).
            wu_src = consts.tile([P, B], BF16)
            nc.gpsimd.memset(wu_src[:], 0.0)
            wu_ps = ps_pool.tile([P, B], F32, tag="ps")
            for i in range(N_WARM):
                nc.tensor.matmul(
                    wu_ps[:], lhsT=wu_src[:, :P], rhs=wu_src[:], start=True, stop=True
                )

            # ---- DMA issue.  Tiny constants (80KB) ride the ACT ring; bulk
            # x on the SP ring, bulk w on the Pool SWDGE ring.  The rings
            # round-robin at packet granularity, so the constants finish
            # fast even against the bulk flood — no hold-back needed.
            ad32_sb = consts.tile([P, KC * M], F32)
            nc.scalar.dma_start(ad32_sb[:], ad32_d.ap())
            gb32_sb = consts.tile([P, GB_W], F32)
            nc.scalar.dma_start(gb32_sb[:], gb32_d.ap())
            xt_tiles = []
            wt_tiles = []
            xdmas = []
            wdmas = []
            k0 = 0
            for g in range(G):
                ks = slice(k0, k0 + GROUP_KCS[g])
                k0 += GROUP_KCS[g]
                xt = xt_pool.tile([P, GROUP_KCS[g], B], BF16, tag="xt")
                xdmas.append(nc.sync.dma_start(xt[:], xh_d.ap()[:, ks, :]))
                wt = wt_pool.tile([P, GROUP_KCS[g], O_CORE], BF16, tag="wt")
                wdmas.append(nc.gpsimd.dma_start(wt[:], wh_d.ap()[:, ks, :]))
                xt_tiles.append(xt)
                wt_tiles.append(wt)
            # The two bulk rings share the SDMA engine pool, and x's head
            # delivery runs ~2x faster than consumption while w's ramps too
            # slowly to make its k=4..15 deadlines.  Stagger x's 1MB tail
            # groups behind w's head completions so w gets the engine share
            # while its deadlines are tight; x's tails still land with
            # >10us of slack.
            for xg, wg in ((4, 1), (5, 4), (6, 5)):
                tile.add_dep_helper(
                    xdmas[xg].ins, wdmas[wg].ins,
                    reason=f"x tail g{xg} yields SDMA share to w head g{wg}",
                )

            # consume the warm-up psum (keeps bacc DCE honest) on the
            # otherwise-idle ACT engine so the DVE stream is untouched;
            # this also frees the wu PSUM bank for phase-1's 8th chain.
            wu_sink = consts.tile([P, 32], F32)
            nc.scalar.activation(wu_sink[:], wu_ps[:, :32], AF.Copy)

            g_v = gb32_sb[:, GB_G:GB_B].rearrange("p (o m) -> p o m", m=M)
            b_v = gb32_sb[:, GB_B:GB_W].rearrange("p (o m) -> p o m", m=M)

            def epilogue(ps, oc, m, name, on_dve=False):
                osb = out_pool.tile([P, B], BF16, tag="osb", name=name)
                if on_dve:
                    # fused (psum * gamma) + bias on the DVE (~2x faster
                    # than ACT) — used for the tail-exposed final slot
                    nc.vector.tensor_scalar(
                        osb[:], ps[:],
                        g_v[:, oc, m : m + 1], b_v[:, oc, m : m + 1],
                        mybir.AluOpType.mult, mybir.AluOpType.add,
                    )
                else:
                    nc.scalar.activation(
                        osb[:],
                        ps[:],
                        AF.Identity,
                        bias=b_v[:, oc, m : m + 1],
                        scale=g_v[:, oc, m : m + 1],
                    )
                nc.sync.dma_start(
                    out_d.ap()[oc * P : (oc + 1) * P, m * B : (m + 1) * B],
                    osb[:],
                )

            # ---- main GEMM: two m-phases, 8 PSUM chains each.
            xab1_ref = {}
            for ph, ms in enumerate(((0, 1), (2, 3))):
                pss = {}
                for m in ms:
                    for oc in range(OC):
                        pss[(oc, m)] = ps_pool.tile(
                            [P, B], F32, tag="ps", name=f"ps{ph}_{m}_{oc}"
                        )
                xabs = {}
                if ph == 0:
                    # k-outer, all 8 chains interleaved: consumption tracks
                    # the bulk-DMA arrival rate without stalling.
                    for k in range(KC):
                        g, j = GROUP_OF_K[k]
                        for m in ms:
                            xab = xab1_pool.tile(
                                [P, B], BF16, tag="xab1", name=f"xa{ph}_{k}_{m}"
                            )
                            op = nc.vector.tensor_scalar_mul(
                                xab[:], xt_tiles[g][:, j, :],
                                ad32_sb[:, k * M + m : k * M + m + 1],
                            )
                            xab1_ref[k] = op
                            xabs[(k, m)] = xab
                        for oc in range(OC):
                            for m in ms:
                                nc.tensor.matmul(
                                    pss[(oc, m)][:],
                                    lhsT=wt_tiles[g][:, j, oc * P : (oc + 1) * P],
                                    rhs=xabs[(k, m)][:],
                                    start=(k == 0),
                                    stop=(k == KC - 1),
                                )
                else:
                    # all data resident: xab prefetch (dep-pinned behind the
                    # same-k phase-1 xab so it can never head-of-line block
                    # the DVE queue on a missing DMA group; gpsimd was tried
                    # for these and is ~20x slower per op), then
                    # slot-contiguous chains so PSUM bank recycling
                    # pipelines against phase-1 epilogues.
                    for k in range(KC):
                        g, j = GROUP_OF_K[k]
                        for m in ms:
                            xab = xab2_pool.tile(
                                [P, B], BF16, tag="xab2", name=f"xa{ph}_{k}_{m}"
                            )
                            op = nc.vector.tensor_scalar_mul(
                                xab[:], xt_tiles[g][:, j, :],
                                ad32_sb[:, k * M + m : k * M + m + 1],
                            )
                            tile.add_dep_helper(
                                op.ins, xab1_ref[k].ins,
                                reason="phase2 xab after phase1 xab(k)",
                            )
                            xabs[(k, m)] = xab
                    for m in ms:
                        for oc in range(OC):
                            for k in range(KC):
                                g, j = GROUP_OF_K[k]
                                nc.tensor.matmul(
                                    pss[(oc, m)][:],
                                    lhsT=wt_tiles[g][:, j, oc * P : (oc + 1) * P],
                                    rhs=xabs[(k, m)][:],
                                    start=(k == 0),
                                    stop=(k == KC - 1),
                                )
                # epilogues in the same slot order phase 2 allocates PSUM,
                # so the pool ring pairs each phase-2 chain with the
                # phase-1 chain whose epilogue it waits on.
                for m in ms:
                    for oc in range(OC):
                        last = ph == 1 and m == ms[-1] and oc == OC - 1
                        epilogue(pss[(oc, m)], oc, m, f"o{ph}_{m}_{oc}",
                                 on_dve=last)

    nc.compile()
    return nc


def _get_nc():
    if "nc" not in _nc_cache:
        _nc_cache["nc"] = _build_nc()
    return _nc_cache["nc"]


def _pk(a2d):
    """(C*P, W) -> (P, C*W): row 128c+p -> [p, c, :] flattened."""
    c = a2d.shape[0] // P
    w = a2d.shape[1]
    return np.ascontiguousarray(
        a2d.reshape(c, P, w).transpose(1, 0, 2).reshape(P, c * w)
    )


def kernel(
    x, eps, alpha, gamma, bias_p, fc_w,
    enc1_w, enc1_b, encm_w, encm_b, dec_w, dec_b,
):
    bf16 = ml_dtypes.bfloat16
    f32 = np.float32
    asc = np.ascontiguousarray

    x = np.asarray(x, f32)
    fc_w = np.asarray(fc_w, f32)

    # ---- fold the tiny VAE encoder into input prep (fp32, exact)
    alpha = np.asarray(alpha, f32)
    emb = np.maximum(alpha @ np.asarray(enc1_w, f32).T + np.asarray(enc1_b, f32), 0.0)
    mu = emb @ np.asarray(encm_w, f32).T + np.asarray(encm_b, f32)
    z = np.asarray(eps, f32) * np.exp(0.5 * mu) + mu
    adec = z @ np.asarray(dec_w, f32).T + np.asarray(dec_b, f32)   # (M, IN)

    # x: (B, IN) -> xh (P, KC, B) bf16, xh[p,k,r] = x[r, 128k+p]
    xh = asc(x.astype(bf16).T.reshape(KC, P, B).transpose(1, 0, 2))
    # fc_w: (OUT, IN) -> per-core wh (P, KC, O_CORE) bf16
    wT_full = fc_w.astype(bf16).T  # (IN, OUT) view

    ad32 = _pk(asc(adec.T))                               # (P, KC*M) f32

    gT_full = np.asarray(gamma, f32).T                    # (OUT, M)
    bT_full = np.asarray(bias_p, f32).T                   # (OUT, M)

    in_maps = []
    for c in range(N_CORES):
        o0, o1 = c * O_CORE, (c + 1) * O_CORE
        wh = asc(wT_full[:, o0:o1].reshape(KC, P, O_CORE).transpose(1, 0, 2))
        gb32 = np.empty((P, GB_W), f32)
        gb32[:, GB_G:GB_B] = _pk(asc(gT_full[o0:o1]))
        gb32[:, GB_B:GB_W] = _pk(asc(bT_full[o0:o1]))
        in_maps.append({"xh": xh, "wh": wh, "ad32": ad32, "gb32": gb32})

    nc = _get_nc()
    res = None
    for attempt in range(3):
        try:
            res = run_bass_kernel_spmd(nc, in_maps, list(range(N_CORES)))
            break
        except Exception:
            # transient NRT_EXEC_UNIT_UNRECOVERABLE wedges can follow an
            # earlier crashed process on the same cores; retry clears it
            if attempt == 2:
                raise
            import time

            time.sleep(5.0)
    outT = np.concatenate(
        [np.asarray(res.results[c]["out"]) for c in range(N_CORES)], axis=0
    )  # (OUT, M*B) bf16
    return asc(outT.T.astype(np.float32))  # (M*B, OUT)


# revision 30
# speedup vs baseline: 4.0952x; 1.0103x over previous
"""Trainium2 Bass kernel for nn_Ensemble_FC (BatchEnsemble fully-connected layer).

Math (reference):
    emb   = relu(alpha @ enc1_w.T + enc1_b)          # (M, H)
    mu    = emb @ encm_w.T + encm_b                  # (M, H)
    z     = eps * exp(0.5 * mu) + mu
    adec  = z @ dec_w.T + dec_b                      # (M, IN)
    out[m*B+i, o] = (sum_k x[i,k] * adec[m,k] * fc_w[o,k]) * gamma[m,o] + bias_p[m,o]

The VAE encoder producing adec is 4x33x4096 ~ 1.1 MFLOP of the 68.7 GFLOP
problem (0.002%); it is folded into host-side input preparation (like the
dec_b / 0.5*encm_b constant folding this kernel always did), so the device
program is the pure BatchEnsemble GEMM.

Sharding: tensor-parallel column-split of fc_w / gamma / bias_p over
out_features (4096 -> 8 x 512).  Every core runs the full (M*B = 2048)-row
GEMM for its 512 output columns.

On-chip layout is transposed ([feature, row]) so per-model scales
(adec, gamma, bias) are per-partition scalars:
    out_core[o_local, m*B+i] = psum * gamma + bias,
    psum = sum_kc  wT[kc, o-chunk].T @ (xT[kc] * adecT[kc, m])
GEMM runs in bf16 (rounded on host), fp32 PSUM accumulation; epilogue
scale+bias in fp32, stored bf16 and upcast on host (tolerance 2e-2).

Perf structure (trace-driven):
- ~7.3us fixed prologue (runtime barriers + IRAM loads) before any user
  instruction runs; warm-up matmuls bridge from there to first data and
  trip the HAM clock gate (cold PE 1.2 GHz -> 2.4 GHz after ~3.4us busy).
- Two m-phases of the GEMM, each k-outer over all 4 output chunks so each
  scaled activation tile xab(k,m) is built ONCE on the DVE and feeds 4
  matmuls (the old 2-pass oc-split built every xab twice and the DVE
  FIFO head-of-line blocked the stream).
- Phase 2 xab prefetch is dep-pinned behind the same-k phase-1 xab so a
  not-yet-landed DMA group can never head-of-line block the DVE queue.
- Phase 2 matmul chains run slot-contiguous (32 MMs per (oc,m)), so PSUM
  bank recycling pipelines against phase-1 epilogues with one ~0.8us
  bubble; phase-1 stays k-outer interleaved to track bulk-DMA arrival.
- Bulk DMA: tiny head groups + few big tail groups (issues are
  semaphore-chained per queue), and x's 1MB tail groups are dep-pinned
  behind w head-group completions — the two rings share the SDMA engine
  pool and the w ring otherwise ramps too slowly for its k=2..15
  deadlines while x runs 2x ahead of demand.
- Measured (healthy 2.4 GHz P-state): ~131us vs the 142us 2-pass
  baseline; stream runs at the 216ns/MM N=512 bf16 floor throughout.
"""

import os
import sys

for _p in ("/opt/trn_rl_repo",):
    if os.path.isdir(_p) and _p not in sys.path:
        sys.path.insert(0, _p)

import numpy as np
import ml_dtypes

import concourse.bass as bass  # noqa: F401  (registers engine libraries)
import concourse.mybir as mybir
import concourse.tile as tile
from concourse import bacc
from concourse.bass_utils import run_bass_kernel_spmd

N_CORES = 8
M = 4          # ensemble members
B = 512        # batch
IN = 4096      # in_features (contraction)
OUT = 4096     # out_features
H = 32         # encoder hidden
P = 128        # partitions
KC = IN // P   # 32 contraction chunks of 128
O_CORE = OUT // N_CORES   # 512 output columns per core
OC = O_CORE // P          # 4 o-chunks of 128 per core
N_WARM = 10    # PE warm-up matmuls bridging the prologue to first data;
               # sized so the PE is continuously busy from the prologue
               # into the stream (any idle gap resets the HAM busy window
               # and the first ~12 stream matmuls run at 1.2 GHz)

# bulk-stream DMA groups (kc each); small head groups so the first
# matmuls aren't gated on a full 512KB transfer
# issues are semaphore-chained per queue (~3 in flight), so use few big
# tail groups: small heads start the stream early, big tails keep the
# issue pipeline from gating delivery
GROUP_KCS = [1, 1, 2, 4, 8, 8, 8]
G = len(GROUP_KCS)
GROUP_OF_K = []
for _g, _n in enumerate(GROUP_KCS):
    GROUP_OF_K += [(_g, _j) for _j in range(_n)]

# gb32 column layout (f32, [128, GB_W])
GB_G = 0                      # [p, oc, m]  OC*M = 16
GB_B = GB_G + OC * M
GB_W = GB_B + OC * M          # 32

F32 = mybir.dt.float32
BF16 = mybir.dt.bfloat16
AF = mybir.ActivationFunctionType

_nc_cache = {}


def _build_nc():
    """Build and compile the per-core Bass/Tile program (SPMD, same on all 8)."""
    nc = bacc.Bacc("TRN2", num_devices=N_CORES, debug=False)

    xh_d = nc.declare_dram_parameter("xh", [P, KC, B], BF16, isOutput=False)
    wh_d = nc.declare_dram_parameter("wh", [P, KC, O_CORE], BF16, isOutput=False)
    ad32_d = nc.declare_dram_parameter("ad32", [P, KC * M], F32, isOutput=False)
    gb32_d = nc.declare_dram_parameter("gb32", [P, GB_W], F32, isOutput=False)
    out_d = nc.declare_dram_parameter("out", [O_CORE, M * B], BF16, isOutput=True)

    with tile.TileContext(nc) as tc:
        with (
            tc.tile_pool(name="consts", bufs=1) as consts,
            tc.tile_pool(name="xt", bufs=G) as xt_pool,
            tc.tile_pool(name="wt", bufs=G) as wt_pool,
            tc.tile_pool(name="xab1", bufs=16) as xab1_pool,
            tc.tile_pool(name="xab2", bufs=2 * KC) as xab2_pool,
            tc.tile_pool(name="ps", bufs=8, space="PSUM") as ps_pool,
            tc.tile_pool(name="osb", bufs=6) as out_pool,
        ):
            # ---- PE warm-up: garbage matmuls bridge the prologue + DMA
            # latency and trip the HAM activity monitor (1.2 -> 2.4 GHz).
            wu_src = consts.tile([P, B], BF16)
            nc.gpsimd.memset(wu_src[:], 0.0)
            wu_ps = ps_pool.tile([P, B], F32, tag="ps")
            for i in range(N_WARM):
                nc.tensor.matmul(
                    wu_ps[:], lhsT=wu_src[:, :P], rhs=wu_src[:], start=True, stop=True
                )

            # ---- DMA issue.  Tiny constants (80KB) ride the ACT ring; bulk
            # x on the SP ring, bulk w on the Pool SWDGE ring.  The rings
            # round-robin at packet granularity, so the constants finish
            # fast even against the bulk flood — no hold-back needed.
            ad32_sb = consts.tile([P, KC * M], F32)
            nc.scalar.dma_start(ad32_sb[:], ad32_d.ap())
            gb32_sb = consts.tile([P, GB_W], F32)
            nc.scalar.dma_start(gb32_sb[:], gb32_d.ap())
            xt_tiles = []
            wt_tiles = []
            xdmas = []
            wdmas = []
            k0 = 0
            for g in range(G):
                ks = slice(k0, k0 + GROUP_KCS[g])
                k0 += GROUP_KCS[g]
                xt = xt_pool.tile([P, GROUP_KCS[g], B], BF16, tag="xt")
                xdmas.append(nc.sync.dma_start(xt[:], xh_d.ap()[:, ks, :]))
                wt = wt_pool.tile([P, GROUP_KCS[g], O_CORE], BF16, tag="wt")
                wdmas.append(nc.gpsimd.dma_start(wt[:], wh_d.ap()[:, ks, :]))
                xt_tiles.append(xt)
                wt_tiles.append(wt)
            # The two bulk rings share the SDMA engine pool, and x's head
            # delivery runs ~2x faster than consumption while w's ramps too
            # slowly to make its k=4..15 deadlines.  Stagger x's 1MB tail
            # groups behind w's head completions so w gets the engine share
            # while its deadlines are tight; x's tails still land with
            # >10us of slack.
            for xg, wg in ((4, 1), (5, 4), (6, 5)):
                tile.add_dep_helper(
                    xdmas[xg].ins, wdmas[wg].ins,
                    reason=f"x tail g{xg} yields SDMA share to w head g{wg}",
                )

            # consume the warm-up psum (keeps bacc DCE honest) on the
            # otherwise-idle ACT engine so the DVE stream is untouched;
            # this also frees the wu PSUM bank for phase-1's 8th chain.
            wu_sink = consts.tile([P, 32], F32)
            nc.scalar.activation(wu_sink[:], wu_ps[:, :32], AF.Copy)

            g_v = gb32_sb[:, GB_G:GB_B].rearrange("p (o m) -> p o m", m=M)
            b_v = gb32_sb[:, GB_B:GB_W].rearrange("p (o m) -> p o m", m=M)

            def epilogue(ps, oc, m, name, on_dve=False):
                osb = out_pool.tile([P, B], BF16, tag="osb", name=name)
                if on_dve:
                    # fused (psum * gamma) + bias on the DVE (~2x faster
                    # than ACT) — used for the tail-exposed final slot
                    nc.vector.tensor_scalar(
                        osb[:], ps[:],
                        g_v[:, oc, m : m + 1], b_v[:, oc, m : m + 1],
                        mybir.AluOpType.mult, mybir.AluOpType.add,
                    )
                else:
                    nc.scalar.activation(
                        osb[:],
                        ps[:],
                        AF.Identity,
                        bias=b_v[:, oc, m : m + 1],
                        scale=g_v[:, oc, m : m + 1],
                    )
                nc.sync.dma_start(
                    out_d.ap()[oc * P : (oc + 1) * P, m * B : (m + 1) * B],
                    osb[:],
                )

            # ---- main GEMM: two m-phases, 8 PSUM chains each.
            xab1_ref = {}
            for ph, ms in enumerate(((0, 1), (2, 3))):
                pss = {}
                for m in ms:
                    for oc in range(OC):
                        pss[(oc, m)] = ps_pool.tile(
                            [P, B], F32, tag="ps", name=f"ps{ph}_{m}_{oc}"
                        )
                xabs = {}
                if ph == 0:
                    # k-outer, all 8 chains interleaved: consumption tracks
                    # the bulk-DMA arrival rate without stalling.
                    for k in range(KC):
                        g, j = GROUP_OF_K[k]
                        for m in ms:
                            xab = xab1_pool.tile(
                                [P, B], BF16, tag="xab1", name=f"xa{ph}_{k}_{m}"
                            )
                            op = nc.vector.tensor_scalar_mul(
                                xab[:], xt_tiles[g][:, j, :],
                                ad32_sb[:, k * M + m : k * M + m + 1],
                            )
                            xab1_ref[k] = op
                            xabs[(k, m)] = xab
                        for oc in range(OC):
                            for m in ms:
                                nc.tensor.matmul(
                                    pss[(oc, m)][:],
                                    lhsT=wt_tiles[g][:, j, oc * P : (oc + 1) * P],
                                    rhs=xabs[(k, m)][:],
                                    start=(k == 0),
                                    stop=(k == KC - 1),
                                )
                else:
                    # all data resident: every phase-2 xab is dep-pinned
                    # behind the LAST phase-1 xab, so none can interleave
                    # into (or HOL-block) the live phase-1 DVE stream —
                    # per-k pins serialized the pipelined DVE every ~25 ops
                    # and cost a matmul slot each ~10.8us.  By the time the
                    # pin resolves all bulk groups landed, the DVE is idle,
                    # and its 262ns/op production still beats the first
                    # chain's 216ns/tile consumption with >1.5us to spare
                    # (gpsimd was tried for these: ~20x slower per op).
                    # Chains run slot-contiguous so PSUM bank recycling
                    # pipelines against phase-1 epilogues.
                    for k in range(KC):
                        g, j = GROUP_OF_K[k]
                        for m in ms:
                            xab = xab2_pool.tile(
                                [P, B], BF16, tag="xab2", name=f"xa{ph}_{k}_{m}"
                            )
                            op = nc.vector.tensor_scalar_mul(
                                xab[:], xt_tiles[g][:, j, :],
                                ad32_sb[:, k * M + m : k * M + m + 1],
                            )
                            tile.add_dep_helper(
                                op.ins, xab1_ref[KC - 1].ins,
                                reason="phase2 xab after final phase1 xab",
                            )
                            xabs[(k, m)] = xab
                    for m in ms:
                        for oc in range(OC):
                            for k in range(KC):
                                g, j = GROUP_OF_K[k]
                                nc.tensor.matmul(
                                    pss[(oc, m)][:],
                                    lhsT=wt_tiles[g][:, j, oc * P : (oc + 1) * P],
                                    rhs=xabs[(k, m)][:],
                                    start=(k == 0),
                                    stop=(k == KC - 1),
                                )
                # epilogues in the same slot order phase 2 allocates PSUM,
                # so the pool ring pairs each phase-2 chain with the
                # phase-1 chain whose epilogue it waits on.
                for m in ms:
                    for oc in range(OC):
                        last = ph == 1 and m == ms[-1] and oc == OC - 1
                        epilogue(pss[(oc, m)], oc, m, f"o{ph}_{m}_{oc}",
                                 on_dve=last)

    nc.compile()
    return nc


def _get_nc():
    if "nc" not in _nc_cache:
        _nc_cache["nc"] = _build_nc()
    return _nc_cache["nc"]


def _pk(a2d):
    """(C*P, W) -> (P, C*W): row 128c+p -> [p, c, :] flattened."""
    c = a2d.shape[0] // P
    w = a2d.shape[1]
    return np.ascontiguousarray(
        a2d.reshape(c, P, w).transpose(1, 0, 2).reshape(P, c * w)
    )


def kernel(
    x, eps, alpha, gamma, bias_p, fc_w,
    enc1_w, enc1_b, encm_w, encm_b, dec_w, dec_b,
):
    bf16 = ml_dtypes.bfloat16
    f32 = np.float32
    asc = np.ascontiguousarray

    x = np.asarray(x, f32)
    fc_w = np.asarray(fc_w, f32)

    # ---- fold the tiny VAE encoder into input prep (fp32, exact)
    alpha = np.asarray(alpha, f32)
    emb = np.maximum(alpha @ np.asarray(enc1_w, f32).T + np.asarray(enc1_b, f32), 0.0)
    mu = emb @ np.asarray(encm_w, f32).T + np.asarray(encm_b, f32)
    z = np.asarray(eps, f32) * np.exp(0.5 * mu) + mu
    adec = z @ np.asarray(dec_w, f32).T + np.asarray(dec_b, f32)   # (M, IN)

    # x: (B, IN) -> xh (P, KC, B) bf16, xh[p,k,r] = x[r, 128k+p]
    xh = asc(x.astype(bf16).T.reshape(KC, P, B).transpose(1, 0, 2))
    # fc_w: (OUT, IN) -> per-core wh (P, KC, O_CORE) bf16
    wT_full = fc_w.astype(bf16).T  # (IN, OUT) view

    ad32 = _pk(asc(adec.T))                               # (P, KC*M) f32

    gT_full = np.asarray(gamma, f32).T                    # (OUT, M)
    bT_full = np.asarray(bias_p, f32).T                   # (OUT, M)

    in_maps = []
    for c in range(N_CORES):
        o0, o1 = c * O_CORE, (c + 1) * O_CORE
        wh = asc(wT_full[:, o0:o1].reshape(KC, P, O_CORE).transpose(1, 0, 2))
        gb32 = np.empty((P, GB_W), f32)
        gb32[:, GB_G:GB_B] = _pk(asc(gT_full[o0:o1]))
        gb32[:, GB_B:GB_W] = _pk(asc(bT_full[o0:o1]))
        in_maps.append({"xh": xh, "wh": wh, "ad32": ad32, "gb32": gb32})

    nc = _get_nc()
    res = None
    for attempt in range(3):
        try:
            res = run_bass_kernel_spmd(nc, in_maps, list(range(N_CORES)))
            break
        except Exception:
            # transient NRT_EXEC_UNIT_UNRECOVERABLE wedges can follow an
            # earlier crashed process on the same cores; retry clears it
            if attempt == 2:
                raise
            import time

            time.sleep(5.0)
    outT = np.concatenate(
        [np.asarray(res.results[c]["out"]) for c in range(N_CORES)], axis=0
    )  # (OUT, M*B) bf16
    return asc(outT.T.astype(np.float32))  # (M*B, OUT)
